# revision 21
# baseline (speedup 1.0000x reference)
"""Distributed GCN encoder (2x spmm+linear) on 8 Trainium2 NeuronCores — v4.

Strategy: partition destination nodes contiguously across the 8 cores;
each core owns the edges whose destination is local. spmm and the dense
Linear commute, so each layer is: dense projection -> gather projected rows
per edge -> one-hot-matmul segment reduce in PSUM.

v4 notes (HW-measured, vs the v2 baseline at 3.86ms -> 3.76ms):
- S built by two DVE broadcast tensor_tensor ops per (superblock, chunk)
  unit (SBUILD=tt). Fine-grained alternatives measured far slower on HW:
  per-tile tensor_scalar ~9.9ms, gpsimd local_scatter ~10.6ms — per-
  instruction/per-call overheads dominate (HW >> cost model).
- SBB=4 dest blocks per superblock. SBB=8 measured 4.76ms (+1ms): bigger
  units pipeline worse despite fewer gather instructions.
- dma_gather single_packet=True (K_SP=1) crashes the runtime; keep 0.
- K_BAL=1 (host-side balanced dst assignment, ~2% fewer padded tiles)
  measured neutral-to-slower; default off.
- (s,c) runs padded to an even tile count (local_scatter legacy; ~0.8%).
- Gather cost is descriptor-count-bound (~2.2ns/row): sequential-index
  gathers measured within 3% of random — locality does not matter.

v5 (K_AGS2=1, default): the monolithic AllGather is split into one
collective per superblock writing a slab-major L2 table ([sb][core][rows]
layout keeps each collective's output contiguous for the BIR verifier).
Phase C uses a second edge grid built over slab-space chunks. The gather
stream flows through the B->C boundary instead of draining behind a
~270us collective: 3.76-3.99ms -> 3.41ms measured.
"""
import os
import sys

sys.path.insert(0, "/opt/trn_rl_repo")

import numpy as np

NCORES = 8
CHUNK_MAX = 25000  # dma_gather idx is int16; chunk the gather table
BLK = 128          # dest nodes per PSUM block (= matmul N)
SW = 14            # local_scatter window (tiles); 14*128*32 < 2^16, even

LAST_RESULT = None  # BassKernelResults of the most recent run (for test.py)


def kernel(x, adj_rows, adj_cols, adj_vals, W1, b1, W2, b2):
    return _run(
        np.asarray(x, np.float32),
        np.asarray(adj_rows, np.int32),
        np.asarray(adj_cols, np.int32),
        np.asarray(adj_vals, np.float32),
        np.asarray(W1, np.float32),
        np.asarray(b1, np.float32),
        np.asarray(W2, np.float32),
        np.asarray(b2, np.float32),
    )


def _pack_idx16(idx):
    # dma_gather idxs layout: linear k -> [16*g + k%16, k//16], replicated
    # across the 8 groups of 16 partitions so any SWDGE queue's Q7 pair
    # reads its copy.
    n = idx.shape[0]
    a = idx.astype(np.int16).reshape(n // 16, 16).T
    return np.tile(a, (8, 1))


def _balance_perm(rows, cols, N, NLOC, NBLK, NCHUNK, CHUNK):
    """Permute nodes within each core so per-(block,chunk) in-degree stays
    under the per-tile-count cap (ceil to 128s drives the padding). Returns
    pos[node] = permuted global position. An edge's src chunk is invariant
    under within-core permutation (each chunk = 2 whole cores), so there is
    no circular dependency."""
    ch = cols // CHUNK
    d = np.zeros((N, NCHUNK), np.int32)
    np.add.at(d, (rows, ch), 1)
    pos = np.empty(N, np.int64)
    cap_last = NLOC - (NBLK - 1) * BLK  # last block may be partial
    big = np.iinfo(np.int64).max
    for c in range(NCORES):
        nodes = np.arange(c * NLOC, (c + 1) * NLOC)
        dv = d[nodes].astype(np.int64)  # [NLOC, NCHUNK]
        # cap: lowest multiple of BLK covering the all-block mean cell load
        mean_bc = dv.sum() / NBLK / NCHUNK
        base_cap = -(-int(np.ceil(mean_bc)) // BLK) * BLK  # 1021.6 -> 1024
        # overflow valves: block c gets +2 tiles of cap in chunk c, so the
        # unavoidable tail concentrates in aligned cells across cores
        cap_bc = np.full((NBLK, NCHUNK), base_cap, np.int64)
        for cc in range(NCHUNK):
            cap_bc[cc, cc] += 2 * BLK
        order = np.argsort(-dv.max(1), kind="stable")
        load = np.zeros((NBLK, NCHUNK), np.int64)
        cnt = np.zeros(NBLK, np.int64)
        cap = np.full(NBLK, BLK, np.int64)
        cap[NBLK - 1] = cap_last
        slot_blk = np.empty(NLOC, np.int64)
        greedy_cap = cap_bc - 10  # headroom for the low-degree tail
        for i in order:
            dd = dv[i]
            nl = load + dd[None, :]
            over = np.maximum(nl - greedy_cap, 0).sum(1)
            score = over * (1 << 20) + nl.max(1)
            score[cnt >= cap] = big
            b = int(np.argmin(score))
            slot_blk[i] = b
            load[b] += dd
            cnt[b] += 1
        # repair by 1-for-1 swaps (all blocks are full, moves can't happen)
        members = [list(np.nonzero(slot_blk == b)[0]) for b in range(NBLK)]

        def try_fix_cell(b, wc):
            mb = np.array(members[b])
            donors = mb[np.argsort(-dv[mb, wc])[:8]]
            rcv = np.argsort(load[:, wc])
            old_b = np.maximum(load[b] - cap_bc, 0).sum()
            for u in donors:
                du = dv[u]
                for b2 in rcv[:24]:
                    if b2 == b:
                        continue
                    m2 = np.array(members[b2])
                    old_2 = np.maximum(load[b2] - cap_bc, 0).sum()
                    vs = m2[np.argsort(dv[m2, wc])[:6]]
                    for v in vs:
                        dvv = dv[v]
                        nl_b = load[b] - du + dvv
                        nl_2 = load[b2] - dvv + du
                        new = (
                            np.maximum(nl_b - cap_bc, 0).sum()
                            + np.maximum(nl_2 - cap_bc, 0).sum()
                        )
                        if new < old_b + old_2:
                            load[b] = nl_b
                            load[b2] = nl_2
                            members[b].remove(u)
                            members[b].append(v)
                            members[b2].remove(v)
                            members[b2].append(u)
                            slot_blk[u] = b2
                            slot_blk[v] = b
                            return True
            return False

        for _sweep in range(25):
            overflow = np.maximum(load - cap_bc, 0)
            if overflow.sum() == 0:
                break
            cells = np.argwhere(overflow > 0)
            cells = cells[np.argsort(-overflow[cells[:, 0], cells[:, 1]])]
            any_fix = False
            for b, wc in cells:
                while load[b, wc] > cap_bc[b, wc]:
                    if not try_fix_cell(int(b), int(wc)):
                        break
                    any_fix = True
            if not any_fix:
                break
        # assign slots within blocks
        slot = np.empty(NLOC, np.int64)
        nxt = np.zeros(NBLK, np.int64)
        for i in range(NLOC):
            b = slot_blk[i]
            slot[i] = b * BLK + nxt[b]
            nxt[b] += 1
        pos[nodes] = c * NLOC + slot
    return pos


def _preprocess(rows, cols, vals, N, NLOC, NBLK, NCHUNK, CHUNK, SBB):
    """Sort/pad edges into the shared (superblock, chunk, block) tile grid."""
    NSB = (NBLK + SBB - 1) // SBB
    core = rows // NLOC
    rloc = rows - core * NLOC
    blk = rloc // BLK
    ch = cols // CHUNK
    gid = blk * NCHUNK + ch  # group id

    sb_blocks = [list(range(s * SBB, min((s + 1) * SBB, NBLK))) for s in range(NSB)]

    # global group order: sb-major, then chunk, then block
    order_pos = np.empty(NBLK * NCHUNK, np.int64)
    seq = []
    for s in range(NSB):
        for c in range(NCHUNK):
            for b in sb_blocks[s]:
                seq.append(b * NCHUNK + c)
    seq = np.array(seq, np.int64)
    order_pos[seq] = np.arange(len(seq))

    # per-core group counts -> shared tile counts
    counts = np.zeros((NCORES, NBLK * NCHUNK), np.int64)
    for c in range(NCORES):
        m = core == c
        counts[c] = np.bincount(gid[m], minlength=NBLK * NCHUNK)
    T = (counts.max(0) + BLK - 1) // BLK  # tiles per group (shared)

    # pad each (s, c) run to an even tile count (local_scatter num_idxs even)
    for s in range(NSB):
        for c in range(NCHUNK):
            gids = [b * NCHUNK + c for b in sb_blocks[s]]
            if sum(int(T[g]) for g in gids) % 2:
                T[gids[-1]] += 1

    T_seq = T[seq]
    tile_base_seq = np.concatenate([[0], np.cumsum(T_seq)])
    NT = int(tile_base_seq[-1])
    tile_base = np.empty(NBLK * NCHUNK, np.int64)
    tile_base[seq] = tile_base_seq[:-1]

    # (s, c) runs: start tile and tile count
    run_start = np.zeros((NSB, NCHUNK), np.int64)
    run_nt = np.zeros((NSB, NCHUNK), np.int64)
    for s in range(NSB):
        for c in range(NCHUNK):
            gids = [b * NCHUNK + c for b in sb_blocks[s]]
            run_start[s, c] = tile_base[gids[0]]
            run_nt[s, c] = int(sum(T[g] for g in gids))

    # scatter-window offset per global tile: (t - run_start) % SW
    w_off = np.zeros(NT, np.int64)
    for s in range(NSB):
        for c in range(NCHUNK):
            t0, nt = int(run_start[s, c]), int(run_nt[s, c])
            w_off[t0 : t0 + nt] = np.arange(nt) % SW

    per_core = []
    for c in range(NCORES):
        m = core == c
        ec, er, ev, eg = cols[m], rloc[m], vals[m], gid[m]
        o = np.lexsort((ec, order_pos[eg]))
        ec, er, ev, eg = ec[o], er[o], ev[o], eg[o]
        cnt = counts[c]
        starts_per_group = np.concatenate([[0], np.cumsum(cnt[seq])])[:-1]
        g_start = np.empty(NBLK * NCHUNK, np.int64)
        g_start[seq] = starts_per_group
        rank = np.arange(len(ec)) - g_start[eg]
        slot = tile_base[eg] * BLK + rank
        tile_of_slot = slot // BLK

        idx_arr = np.zeros(NT * BLK, np.int16)
        sidx_arr = np.full(NT * BLK, -1, np.int16)
        rows_arr = np.full(NT * BLK, -1.0, np.float32)
        vals_arr = np.zeros(NT * BLK, np.float32)
        idx_arr[slot] = (ec - (ec // CHUNK) * CHUNK).astype(np.int16)
        sidx_arr[slot] = (
            w_off[tile_of_slot] * BLK + (er - (er // BLK) * BLK)
        ).astype(np.int16)
        rows_arr[slot] = (er - (er // BLK) * BLK).astype(np.float32)
        vals_arr[slot] = ev
        per_core.append(
            (
                _pack_idx16(idx_arr),
                np.ascontiguousarray(sidx_arr.reshape(NT, BLK).T),
                np.ascontiguousarray(rows_arr.reshape(NT, BLK).T),
                np.ascontiguousarray(vals_arr.reshape(NT, BLK).T),
            )
        )
    return T, tile_base, NT, per_core, sb_blocks, run_start, run_nt


def _run(x, adj_rows, adj_cols, adj_vals, W1, b1, W2, b2, trace=None):
    global LAST_RESULT
    import concourse.bacc as bacc
    import concourse.mybir as mybir
    import concourse.tile as tile
    from concourse import bass_utils

    N, F0 = x.shape
    F1 = W1.shape[1]
    F2 = W2.shape[1]
    assert N % NCORES == 0
    NLOC = N // NCORES
    NBLK = (NLOC + BLK - 1) // BLK
    SBB = int(os.environ.get("K_SBB", "4"))
    NSB = (NBLK + SBB - 1) // SBB
    NCHUNK = (N + CHUNK_MAX - 1) // CHUNK_MAX
    CHUNK = (N + NCHUNK - 1) // NCHUNK
    assert CHUNK <= 32768

    # --- host-side node permutation (balanced dst assignment) ---
    if os.environ.get("K_BAL", "0") == "1":
        pos = _balance_perm(adj_rows, adj_cols, N, NLOC, NBLK, NCHUNK, CHUNK)
    else:
        pos = np.arange(N, dtype=np.int64)
    inv = np.empty(N, np.int64)
    inv[pos] = np.arange(N)
    prows = pos[adj_rows]
    pcols = pos[adj_cols]
    xp = x[inv]  # x rows in permuted order

    T, tile_base, NT, per_core, sb_blocks, run_start, run_nt = _preprocess(
        prows, pcols, adj_vals, N, NLOC, NBLK, NCHUNK, CHUNK, SBB
    )

    AGS2 = os.environ.get("K_AGS2", "1") == "1"
    if AGS2:
        # slab-major L2 table layout: [sb][core][sb-local rows][F2P], so each
        # per-sb AllGather writes one contiguous slab. Map permuted row -> slab
        # index, regroup edges by slab-space chunks (second grid).
        NSB_h = (NBLK + SBB - 1) // SBB
        len_s = np.minimum(
            NLOC - np.arange(NSB_h) * SBB * BLK, SBB * BLK
        ).astype(np.int64)
        slab_off = np.concatenate([[0], np.cumsum(NCORES * len_s)])[:-1]
        r = np.arange(N, dtype=np.int64)
        k = r // NLOC
        j = r - k * NLOC
        sseg = np.minimum(j // (SBB * BLK), NSB_h - 1)
        l2pos = slab_off[sseg] + k * len_s[sseg] + (j - sseg * SBB * BLK)
        (T2, tile_base2, NT2, per_core2, _sb2, run_start2, run_nt2) = _preprocess(
            prows, l2pos[pcols], adj_vals, N, NLOC, NBLK, NCHUNK, CHUNK, SBB
        )

    b2bc_np = np.tile(b2[None, :], (BLK, SBB))

    f32 = mybir.dt.float32
    bf16 = mybir.dt.bfloat16
    i16 = mybir.dt.int16
    nc = bacc.Bacc(
        "TRN2",
        target_bir_lowering=False,
        debug=False,
        num_devices=NCORES,
        num_swdge_queues=int(os.environ.get("K_NSWQ", "4")),
    )
    NSWQ = nc.num_swdge_queues

    # xT2[p, j, i] = xp[i, p + 128*j] as bf16 (full table, replicated)
    xT2_t = nc.dram_tensor("xT2", [128, F0 // 128, N], bf16, kind="ExternalInput")
    W1_t = nc.dram_tensor("W1", [F0, F1], bf16, kind="ExternalInput")
    b1_t = nc.dram_tensor("b1", [F1, 1], f32, kind="ExternalInput")
    W2_t = nc.dram_tensor("W2", [F1, F2], bf16, kind="ExternalInput")
    b2bc_t = nc.dram_tensor("b2bc", [BLK, SBB * F2], f32, kind="ExternalInput")
    idx_t = nc.dram_tensor("idx16", [128, NT * 8], i16, kind="ExternalInput")
    iota_t = nc.dram_tensor("iota", [BLK, BLK], bf16, kind="ExternalInput")
    sidx_t = nc.dram_tensor("sidxT", [128, NT], i16, kind="ExternalInput")
    rows_t = nc.dram_tensor("rowsT", [128, NT], f32, kind="ExternalInput")
    rows16_t = nc.dram_tensor("rows16T", [128, NT], bf16, kind="ExternalInput")
    vals32_t = nc.dram_tensor("vals32T", [128, NT], f32, kind="ExternalInput")
    vals_t = nc.dram_tensor("valsT", [128, NT], bf16, kind="ExternalInput")
    out_t = nc.dram_tensor("out", [NLOC, F2], f32, kind="ExternalOutput")

    xw1_full = nc.dram_tensor("xw1_full", [N, F1], bf16, kind="Internal")
    F2P = 128  # layer-2 table padded to 128 cols for the 256B gather minimum
    h1w2_bounce = nc.dram_tensor("h1w2_bounce", [NLOC, F2P], bf16, kind="Internal")
    h1w2_full = nc.dram_tensor(
        "h1w2_full", [N, F2P], bf16, kind="Internal", addr_space="Shared"
    )
    if AGS2:
        idx2_t = nc.dram_tensor("idx2", [128, NT2 * 8], i16, kind="ExternalInput")
        rows2_t = nc.dram_tensor("rows2T", [128, NT2], bf16, kind="ExternalInput")
        vals2_t = nc.dram_tensor("vals2T", [128, NT2], bf16, kind="ExternalInput")
        h1w2_slab = nc.dram_tensor(
            "h1w2_slab", [N, F2P], bf16, kind="Internal", addr_space="Shared"
        )

    sb_base = [int(run_start[s, 0]) for s in range(NSB)]
    sb_nt = [int(run_nt[s].sum()) for s in range(NSB)]
    MAXNT = max(int(run_nt[s, c]) for s in range(NSB) for c in range(NCHUNK))
    MAXSBNT = max(sb_nt)
    G1 = dict(T=T, tb=tile_base, rs=run_start, rn=run_nt,
              sb_base=sb_base, sb_nt=sb_nt)
    if AGS2:
        sb_base2 = [int(run_start2[s, 0]) for s in range(NSB)]
        sb_nt2 = [int(run_nt2[s].sum()) for s in range(NSB)]
        MAXNT = max(MAXNT, max(int(run_nt2[s, c]) for s in range(NSB)
                               for c in range(NCHUNK)))
        MAXSBNT = max(MAXSBNT, max(sb_nt2))
        G2 = dict(T=T2, tb=tile_base2, rs=run_start2, rn=run_nt2,
                  sb_base=sb_base2, sb_nt=sb_nt2)

    add = mybir.AluOpType.add
    Relu = mybir.ActivationFunctionType.Relu
    REPEAT = int(os.environ.get("K_REPEAT", "1"))
    SBUILD = os.environ.get("K_SBUILD", "tt")  # tt | ts | ls
    L2VM = os.environ.get("K_L2VM", "0") == "1"  # scale 64-wide msgs, not S
    ROT = os.environ.get("K_ROT", "0") == "1"  # rotate chunk order per sb
    POOLFRAC = int(os.environ.get("K_POOLFRAC", "0"))  # 1/N S units on Pool
    SPOOL = int(os.environ.get("K_SPOOL", "0"))  # every Nth S tile on Pool
    SKIPS = os.environ.get("K_SKIP", "").split(",")
    AGSPLIT = os.environ.get("K_AGSPLIT", "0") == "1"

    with tile.TileContext(nc) as tc:
        with (
            tc.tile_pool(name="consts", bufs=1) as cp,
            tc.tile_pool(name="xt", bufs=3) as xtp,
            tc.tile_pool(name="xw1sb", bufs=3) as xw1p,
            tc.tile_pool(name="edata", bufs=2) as ep,
            tc.tile_pool(name="msgs", bufs=4) as mp,
            tc.tile_pool(name="smat", bufs=2) as sp,
            tc.tile_pool(name="epi", bufs=2) as epi,
            tc.tile_pool(name="ps_a", bufs=2, space="PSUM") as ppa,
            tc.tile_pool(name="ps_agg", bufs=2, space="PSUM") as ppagg,
            tc.tile_pool(name="ps_2", bufs=2, space="PSUM") as pp2,
        ):
            w1_sb = cp.tile([128, (F0 // 128) * F1], bf16, tag="w1")
            for k in range(F0 // 128):
                nc.sync.dma_start(
                    w1_sb[:, k * F1 : (k + 1) * F1], W1_t[k * 128 : (k + 1) * 128, :]
                )
            iota_sb = cp.tile([BLK, BLK], bf16, tag="iota")
            nc.sync.dma_start(iota_sb[:], iota_t.ap())
            w2_sb = cp.tile([F1, F2], bf16, tag="w2")
            nc.sync.dma_start(w2_sb[:], W2_t.ap())
            b1_sb = cp.tile([F1, 1], f32, tag="b1")
            nc.sync.dma_start(b1_sb[:], b1_t.ap())
            b2_sb = cp.tile([BLK, SBB * F2], f32, tag="b2bc")
            nc.sync.dma_start(b2_sb[:], b2bc_t.ap())

            for _rep in range(REPEAT):
                # ---- phase A: xw1_full = xp @ W1, computed replicated on
                # every core, in permuted-row (= chunk) order so layer-1
                # gathers pipeline behind it.
                AROWS = 2 * BLK
                NTA = (N + AROWS - 1) // AROWS
                for i in ([] if "a" in SKIPS else range(NTA)):
                    nr = min(AROWS, N - i * AROWS)
                    xt = xtp.tile([128, F0 // 128, AROWS], bf16, tag="xt")
                    nc.sync.dma_start(
                        xt[:, :, :nr], xT2_t[:, :, i * AROWS : i * AROWS + nr]
                    )
                    for h in range(0, nr, BLK):
                        nb_r = min(BLK, nr - h)
                        ps = ppa.tile([128, F1], f32, tag="psa")
                        for k in range(F0 // 128):
                            nc.tensor.matmul(
                                ps[:nb_r, :],
                                xt[:, k, h : h + nb_r],
                                w1_sb[:, k * F1 : (k + 1) * F1],
                                start=(k == 0),
                                stop=(k == F0 // 128 - 1),
                            )
                        xo = xw1p.tile([128, F1], bf16, tag="xw1")
                        nc.scalar.copy(xo[:nb_r, :], ps[:nb_r, :])
                        nc.sync.dma_start(
                            xw1_full[i * AROWS + h : i * AROWS + h + nb_r, :],
                            xo[:nb_r, :],
                        )

                # ---- shared machinery for phases B and C
                def edge_tiles(s, G=G1, layer2=False):
                    idx_sb = ep.tile([128, MAXSBNT * 8], i16, tag="idx")
                    base, nt = G["sb_base"][s], G["sb_nt"][s]
                    if layer2:
                        nc.sync.dma_start(
                            idx_sb[:, : nt * 8],
                            idx2_t[:, base * 8 : (base + nt) * 8],
                        )
                        vals_sb = ep.tile([128, MAXSBNT], bf16, tag="vals")
                        nc.sync.dma_start(
                            vals_sb[:, :nt], vals2_t[:, base : base + nt]
                        )
                        sidx_sb = ep.tile([128, MAXSBNT], bf16, tag="rows16")
                        nc.sync.dma_start(
                            sidx_sb[:, :nt], rows2_t[:, base : base + nt]
                        )
                        return idx_sb, sidx_sb, vals_sb
                    nc.sync.dma_start(
                        idx_sb[:, : nt * 8], idx_t[:, base * 8 : (base + nt) * 8]
                    )
                    if SBUILD == "ls":
                        vals_sb = ep.tile([128, MAXSBNT], bf16, tag="vals")
                        nc.sync.dma_start(vals_sb[:, :nt], vals_t[:, base : base + nt])
                        sidx_sb = ep.tile([128, MAXSBNT], i16, tag="sidx")
                        nc.sync.dma_start(
                            sidx_sb[:, :nt], sidx_t[:, base : base + nt]
                        )
                    elif SBUILD == "tt":
                        vals_sb = ep.tile([128, MAXSBNT], bf16, tag="vals")
                        nc.sync.dma_start(vals_sb[:, :nt], vals_t[:, base : base + nt])
                        sidx_sb = ep.tile([128, MAXSBNT], bf16, tag="rows16")
                        nc.sync.dma_start(
                            sidx_sb[:, :nt], rows16_t[:, base : base + nt]
                        )
                    else:
                        vals_sb = ep.tile([128, MAXSBNT], f32, tag="vals32")
                        nc.sync.dma_start(
                            vals_sb[:, :nt], vals32_t[:, base : base + nt]
                        )
                        sidx_sb = ep.tile([128, MAXSBNT], f32, tag="rows")
                        nc.sync.dma_start(
                            sidx_sb[:, :nt], rows_t[:, base : base + nt]
                        )
                    return idx_sb, sidx_sb, vals_sb

                def gather_chunk(s, c, off, tsc, msgs_c, idx_sb, table, F):
                    n = tsc * BLK
                    lo = c * CHUNK
                    hi = min((c + 1) * CHUNK, N)
                    if os.environ.get("K_STUB_GATHER"):
                        nc.sync.dma_start(
                            msgs_c[:, :tsc, :],
                            table[lo : lo + n, :].rearrange(
                                "(t p) f -> p t f", p=128
                            ),
                        )
                        return
                    nc.gpsimd.dma_gather(
                        msgs_c[:, :tsc, :],
                        table[lo:hi, :],
                        idx_sb[:, off * 8 : (off + tsc) * 8],
                        n,
                        n,
                        F,
                        single_packet=bool(int(os.environ.get("K_SP", "0"))),
                        queue_num=(s * NCHUNK + c) % NSWQ,
                    )

                def build_S(off, tsc, sidx_sb, vals_sb, with_vals=True,
                            unit=0):
                    S = sp.tile([128, MAXNT, BLK], bf16, tag="S")
                    veng = (
                        nc.gpsimd
                        if POOLFRAC and unit % POOLFRAC == 0
                        else nc.vector
                    )
                    if SBUILD == "ls":
                        w0 = 0
                        while w0 < tsc:
                            wnt = min(SW, tsc - w0)
                            assert wnt % 2 == 0, (tsc, w0)
                            nc.gpsimd.local_scatter(
                                S[:, w0 : w0 + wnt, :],
                                vals_sb[:, off + w0 : off + w0 + wnt],
                                sidx_sb[:, off + w0 : off + w0 + wnt],
                                128,
                                wnt * BLK,
                                wnt,
                            )
                            w0 += wnt
                        return S
                    is_eq = mybir.AluOpType.is_equal
                    mlt = mybir.AluOpType.mult
                    if SBUILD == "tt":
                        from concourse.bass import broadcast_tensor_aps

                        i_bc, r_bc = broadcast_tensor_aps(
                            iota_sb[:][:, None, :],
                            sidx_sb[:, off : off + tsc][:, :, None],
                        )
                        veng.tensor_tensor(S[:, :tsc, :], i_bc, r_bc, is_eq)
                        if with_vals:
                            s_ap, v_bc = broadcast_tensor_aps(
                                S[:, :tsc, :],
                                vals_sb[:, off : off + tsc][:, :, None],
                            )
                            veng.tensor_tensor(S[:, :tsc, :], s_ap, v_bc, mlt)
                        return S
                    for t in range(tsc):
                        eng = (
                            nc.gpsimd
                            if SPOOL and (off + t) % SPOOL == 0
                            else nc.vector
                        )
                        eng.tensor_scalar(
                            S[:, t, :],
                            iota_sb[:],
                            sidx_sb[:, off + t : off + t + 1],
                            vals_sb[:, off + t : off + t + 1],
                            is_eq,
                            mlt,
                        )
                    return S

                def aggregate(s, idx_sb, sidx_sb, vals_sb, table, F, Fm, psum, pw,
                              mtag, G=G1):
                    """Per-chunk gather + S build + one-hot matmuls into psum.

                    F: gathered row width; Fm: matmul width (<= F);
                    pw: psum free-dim width per block; lhsT/rhs roles flip
                    between layers (transposed agg for L1, plain for L2)."""
                    Tg, tbg, rsg, rng = G["T"], G["tb"], G["rs"], G["rn"]
                    tot = {
                        b: int(sum(Tg[b * NCHUNK + c] for c in range(NCHUNK)))
                        for b in sb_blocks[s]
                    }
                    done = {b: 0 for b in sb_blocks[s]}
                    # Interleaved per-slice start groups would lazily re-zero
                    # the whole bank and clobber sibling slices; memset once
                    # and accumulate with start=False throughout.
                    nc.vector.memset(psum[:], 0.0)
                    corder = (
                        [(cc + s) % NCHUNK for cc in range(NCHUNK)]
                        if ROT
                        else list(range(NCHUNK))
                    )
                    for c in corder:
                        tsc = int(rng[s, c])
                        off = int(rsg[s, c] - rsg[s, 0])
                        if tsc == 0:
                            continue
                        msgs_c = mp.tile([128, MAXNT, F], bf16, tag=mtag)
                        if "g" not in SKIPS:
                            gather_chunk(s, c, off, tsc, msgs_c, idx_sb, table, F)
                        l2vm = L2VM and pw != BLK and SBUILD == "tt"
                        if "s" not in SKIPS:
                            S = build_S(off, tsc, sidx_sb, vals_sb,
                                        with_vals=not l2vm,
                                        unit=s * NCHUNK + c)
                            if l2vm:
                                from concourse.bass import broadcast_tensor_aps

                                m_ap, v_bc = broadcast_tensor_aps(
                                    msgs_c[:, :tsc, :Fm],
                                    vals_sb[:, off : off + tsc][:, :, None],
                                )
                                nc.vector.tensor_tensor(
                                    msgs_c[:, :tsc, :Fm], m_ap, v_bc,
                                    mybir.AluOpType.mult,
                                )
                        else:
                            S = sp.tile([128, MAXNT, BLK], bf16, tag="S")
                        if "mm" not in SKIPS:
                            run0 = int(rsg[s, c])
                            for bi, b in enumerate(sb_blocks[s]):
                                g = b * NCHUNK + c
                                t0 = int(tbg[g] - run0)
                                dst = psum[:, bi * pw : (bi + 1) * pw]
                                for j in range(int(Tg[g])):
                                    lt = t0 + j
                                    if pw == BLK:  # L1: psum[f1,d] = msgs.T @ S
                                        lhsT, rhs = msgs_c[:, lt, :], S[:, lt, :]
                                    else:  # L2: psum[d,f2] = S.T @ msgs
                                        lhsT, rhs = S[:, lt, :], msgs_c[:, lt, :Fm]
                                    nc.tensor.matmul(
                                        dst,
                                        lhsT,
                                        rhs,
                                        start=False,
                                        stop=(done[b] == tot[b] - 1),
                                        skip_group_check=True,
                                    )
                                    done[b] += 1

                # ---- phase B: layer-1 gather + segment-reduce + relu + @W2
                hfull_v = h1w2_full.ap().rearrange("(k r) f -> k r f", k=NCORES)
                for s in range(NSB):
                    nb = len(sb_blocks[s])
                    r0 = sb_blocks[s][0] * BLK
                    nrow = min(NLOC, (sb_blocks[s][-1] + 1) * BLK) - r0
                    idx_sb, sidx_sb, vals_sb = edge_tiles(s)
                    psum1 = ppagg.tile([128, SBB * BLK], f32, tag="agg")
                    aggregate(
                        s, idx_sb, sidx_sb, vals_sb, xw1_full, F1, F1, psum1, BLK,
                        "msgs",
                    )

                    h1T = epi.tile([128, SBB * BLK], bf16, tag="h1T")
                    nc.scalar.activation(
                        h1T[:, : nb * BLK], psum1[:, : nb * BLK], Relu,
                        bias=b1_sb[:, 0:1],
                    )
                    psum2 = pp2.tile([128, SBB * F2], f32, tag="ps2")
                    for bi in range(nb):
                        nc.tensor.matmul(
                            psum2[:, bi * F2 : (bi + 1) * F2],
                            h1T[:, bi * BLK : (bi + 1) * BLK],
                            w2_sb[:],
                            start=True,
                            stop=True,
                        )
                    hw = epi.tile([128, SBB * F2P], bf16, tag="hw2")
                    nc.vector.memset(hw[:, : nb * F2P], 0.0)
                    for bi in range(nb):
                        nc.scalar.copy(
                            hw[:, bi * F2P : bi * F2P + F2],
                            psum2[:, bi * F2 : (bi + 1) * F2],
                        )
                    for bi, b in enumerate(sb_blocks[s]):
                        nr = min(BLK, NLOC - b * BLK)
                        nc.sync.dma_start(
                            h1w2_bounce[b * BLK : b * BLK + nr, :],
                            hw[:nr, bi * F2P : (bi + 1) * F2P],
                        )
                    if AGS2 and "ag" not in SKIPS:
                        so = int(slab_off[s])
                        nc.gpsimd.collective_compute(
                            "AllGather",
                            mybir.AluOpType.bypass,
                            replica_groups=[list(range(NCORES))],
                            ins=[h1w2_bounce[r0 : r0 + nrow, :]],
                            outs=[h1w2_slab[so : so + NCORES * nrow, :]],
                        )
                    elif AGSPLIT and "ag" not in SKIPS:
                        nc.gpsimd.collective_compute(
                            "AllGather",
                            mybir.AluOpType.bypass,
                            replica_groups=[list(range(NCORES))],
                            ins=[h1w2_bounce[r0 : r0 + nrow, :]],
                            outs=[hfull_v[:, r0 : r0 + nrow, :]],
                        )

                if not AGSPLIT and not AGS2 and "ag" not in SKIPS:
                    nc.gpsimd.collective_compute(
                        "AllGather",
                        mybir.AluOpType.bypass,
                        replica_groups=[list(range(NCORES))],
                        ins=[h1w2_bounce.ap()],
                        outs=[h1w2_full.ap()],
                    )

                # ---- phase C: layer-2 gather + segment-reduce + bias
                for s in ([] if "l2" in SKIPS else range(NSB)):
                    nb = len(sb_blocks[s])
                    if AGS2:
                        idx_sb, sidx_sb, vals_sb = edge_tiles(s, G2, layer2=True)
                        psum3 = pp2.tile([128, SBB * F2], f32, tag="ps2")
                        aggregate(
                            s, idx_sb, sidx_sb, vals_sb, h1w2_slab, F2P, F2,
                            psum3, F2, "msgs2", G2,
                        )
                    else:
                        idx_sb, sidx_sb, vals_sb = edge_tiles(s)
                        psum3 = pp2.tile([128, SBB * F2], f32, tag="ps2")
                        aggregate(
                            s, idx_sb, sidx_sb, vals_sb, h1w2_full, F2P, F2,
                            psum3, F2, "msgs2",
                        )

                    osb = epi.tile([128, SBB * F2], f32, tag="osb")
                    nc.vector.tensor_tensor(
                        osb[:, : nb * F2], psum3[:, : nb * F2], b2_sb[:, : nb * F2],
                        add,
                    )
                    for bi, b in enumerate(sb_blocks[s]):
                        nr = min(BLK, NLOC - b * BLK)
                        nc.sync.dma_start(
                            out_t[b * BLK : b * BLK + nr, :],
                            osb[:nr, bi * F2 : (bi + 1) * F2],
                        )

    nc.compile()

    import jax.numpy as jnp

    xT2 = np.ascontiguousarray(
        np.transpose(xp.reshape(N, F0 // 128, 128), (2, 1, 0))
    )
    xT2 = np.asarray(jnp.asarray(xT2, dtype=jnp.bfloat16))
    W1b = np.asarray(jnp.asarray(W1, dtype=jnp.bfloat16))
    W2b = np.asarray(jnp.asarray(W2, dtype=jnp.bfloat16))

    iota_np = np.tile(np.arange(BLK, dtype=np.float32)[None, :], (BLK, 1))
    iota_b = np.asarray(jnp.asarray(iota_np, dtype=jnp.bfloat16))
    in_maps = []
    for c in range(NCORES):
        idx16, sidxT, rowsT, valsT = per_core[c]
        in_maps.append(
            {
                "xT2": xT2,
                "W1": W1b,
                "b1": np.ascontiguousarray(b1[:, None]),
                "W2": W2b,
                "b2bc": b2bc_np,
                "idx16": idx16,
                "iota": iota_b,
                "sidxT": sidxT,
                "rowsT": np.ascontiguousarray(rowsT, dtype=np.float32),
                "rows16T": np.asarray(jnp.asarray(rowsT, dtype=jnp.bfloat16)),
                "vals32T": np.ascontiguousarray(valsT, dtype=np.float32),
                "valsT": np.asarray(jnp.asarray(valsT, dtype=jnp.bfloat16)),
            }
        )
        if AGS2:
            idx2, _s2, rows2, vals2 = per_core2[c]
            in_maps[-1]["idx2"] = idx2
            in_maps[-1]["rows2T"] = np.asarray(
                jnp.asarray(rows2, dtype=jnp.bfloat16)
            )
            in_maps[-1]["vals2T"] = np.asarray(
                jnp.asarray(vals2, dtype=jnp.bfloat16)
            )

    bench = int(os.environ.get("K_BENCH", "0"))
    if bench:
        results = _pjrt_bench(nc, in_maps, bench)
    else:
        kwargs = {}
        if trace is not None:
            kwargs["trace"] = trace
        res = bass_utils.run_bass_kernel_spmd(
            nc, in_maps, core_ids=list(range(NCORES)), **kwargs
        )
        LAST_RESULT = res
        results = res.results
    out_cat = np.concatenate([results[c]["out"] for c in range(NCORES)], axis=0)
    return np.ascontiguousarray(out_cat[pos])


LAST_TIMES = None


def _pjrt_bench(nc, in_maps, iters):
    """Replicates bass2jax.run_bass_via_pjrt's multi-core path, with the
    executable built once and timed warm iterations (inputs pre-staged on
    device, zero output-donation buffers made on device)."""
    global LAST_TIMES
    import time

    import jax
    import jax.numpy as jnp
    from jax.sharding import Mesh, NamedSharding, PartitionSpec
    from jax.experimental.shard_map import shard_map

    import concourse.mybir as mybir
    from concourse.bass2jax import (
        _bass_exec_p,
        install_neuronx_cc_hook,
        partition_id_tensor,
    )

    install_neuronx_cc_hook()

    in_names, out_names, out_avals, zero_outs = [], [], [], []
    partition_name = nc.partition_id_tensor.name if nc.partition_id_tensor else None
    for alloc in nc.m.functions[0].allocations:
        if not isinstance(alloc, mybir.MemoryLocationSet):
            continue
        name = alloc.memorylocations[0].name
        if alloc.kind == "ExternalInput":
            if name != partition_name:
                in_names.append(name)
        elif alloc.kind == "ExternalOutput":
            out_names.append(name)
            shape = tuple(alloc.tensor_shape)
            dtype = mybir.dt.np(alloc.dtype)
            out_avals.append(jax.core.ShapedArray(shape, dtype))
            zero_outs.append(np.zeros(shape, dtype))
    n_params = len(in_names)
    n_outs = len(out_avals)
    in_names.extend(out_names)
    if partition_name is not None:
        in_names.append(partition_name)

    def _make_body(chain):
        def _exec(zs, ins):
            operands = ins + list(zs)
            if partition_name is not None:
                operands.append(partition_id_tensor())
            return tuple(
                _bass_exec_p.bind(
                    *operands,
                    out_avals=tuple(out_avals),
                    in_names=tuple(in_names),
                    out_names=tuple(out_names),
                    lowering_input_output_aliases=(),
                    sim_require_finite=True,
                    sim_require_nnan=True,
                    nc=nc,
                )
            )

        def _body(*args):
            ins = list(args[:n_params])
            zs = tuple(args[n_params:])
            if chain == 1:
                return _exec(zs, ins)
            return jax.lax.fori_loop(0, chain, lambda i, z: _exec(z, ins), zs)

        return _body

    ncores = len(in_maps)
    devices = jax.devices()[:ncores]
    mesh = Mesh(np.asarray(devices), ("core",))
    donate = tuple(range(n_params, n_params + n_outs))

    def _make_sharded(chain):
        return jax.jit(
            shard_map(
                _make_body(chain),
                mesh=mesh,
                in_specs=(PartitionSpec("core"),) * (n_params + n_outs),
                out_specs=(PartitionSpec("core"),) * n_outs,
                check_rep=False,
            ),
            donate_argnums=donate,
            keep_unused=True,
        )

    sharded = _make_sharded(1)
    sh = NamedSharding(mesh, PartitionSpec("core"))
    concat_in = [
        np.concatenate([np.asarray(m[in_names[i]]) for m in in_maps], axis=0)
        for i in range(n_params)
    ]
    dev_in = [jax.device_put(a, sh) for a in concat_in]
    zshapes = [(ncores * z.shape[0], *z.shape[1:]) for z in zero_outs]
    zdtypes = [z.dtype for z in zero_outs]
    zeros_maker = jax.jit(
        lambda: tuple(jnp.zeros(s, d) for s, d in zip(zshapes, zdtypes)),
        out_shardings=(sh,) * n_outs,
    )

    def _time_fn(fn, n):
        out, ts = None, []
        for i in range(n + 1):
            zs = jax.block_until_ready(zeros_maker())
            t0 = time.perf_counter()
            cur = jax.block_until_ready(fn(*dev_in, *zs))
            dt = time.perf_counter() - t0
            if i > 0:
                ts.append(dt)
            else:
                out = cur
        return out, ts

    out_arrs, t1 = _time_fn(sharded, iters)
    chain = int(os.environ.get("K_CHAIN", "1"))
    tc = []
    if chain > 1:
        _, tc = _time_fn(_make_sharded(chain), iters)
        per_exec = (min(tc) - min(t1)) / (chain - 1)
        print(
            f"bench: chain1 {min(t1)*1e3:.2f} ms, chain{chain} {min(tc)*1e3:.2f} ms"
            f" -> per-exec {per_exec*1e3:.3f} ms"
        )
    LAST_TIMES = {"t1": t1, "tc": tc, "chain": chain}
    return [
        {
            name: np.asarray(out_arrs[i]).reshape(ncores, *out_avals[i].shape)[c]
            for i, name in enumerate(out_names)
        }
        for c in range(ncores)
    ]
